# revision 1
# baseline (speedup 1.0000x reference)
"""Chamfer distance v2: d-twice, 4-way row-tiled matmuls, split PSUM exit.

Per chunk [128 i x 1024 j] of each per-cluster distance matrix (both
orientations):
  - PE writes the chunk into a PSUM "duo" tile (2 chunks / 4 banks),
    4 row-tiled K=13 matmuls running concurrently (tile_position).
  - ACT casts the two j-half-1 segments of the duo to SBUF bf16 in one
    [128, 2, 512] instruction (PSUM exit via the scalar engine).
  - A custom DVE op (FOLD_MIN_ANT: out = min(in0,in1), accum_out =
    min-reduce) reads j-half-0 from PSUM and the bf16 half from SBUF,
    emitting the chunk's row-min in one 512-cycle pass (PSUM exit via
    DVE at 2 elems/lane/cycle aggregate).
Host sums the [128, 256] per-chunk row-min matrix, masking the top
cluster id, exactly as the reference does.
"""

import numpy as np

C = 128
P = 1024
DIM = 3
K = 13
N_CORES = 8
CPC = C // N_CORES   # 16 clusters per core
ICH = P // 128       # 8 chunks per cluster-direction
OUT_COLS = 2 * CPC * ICH  # 256

_cache = {}


def _get_fold_min_op():
    """Register (once) a custom DVE op: out = min(in0, in1);
    accum_out = min-reduce(out, init=s0)."""
    from concourse.dve_spec import Spec, Src0, Src1, C0, minn
    from concourse import dve_ops as dvo
    from concourse.dve_table_gen import dve_ver_for

    name = "FOLD_MIN_ANT"
    for op in dvo.OPS:
        if op.name == name:
            return op
    op = dvo.DveOp(
        name,
        Spec(body=minn(Src0, Src1), accum=minn, accum_init=C0),
        subdim=False,
        uops_sha={},
    )
    dvo.OPS.append(op)
    dvo.CUSTOM_DVE_SPECS[name] = op.spec
    dvo._SUB_OPCODE_FOR_NAME[name] = max(dvo._SUB_OPCODE_FOR_NAME.values()) + 1
    ver = dve_ver_for("TRN2")
    try:
        op.compile(ver)
    except ValueError as e:
        got = str(e).split(f"{ver}: ")[1].split(" ≠")[0].strip()
        op.uops_sha[ver] = got
    op.compile(ver)
    return op


def _build():
    import concourse.bacc as bacc
    import concourse.mybir as mybir
    from concourse.tile import TileContext

    fold_min = _get_fold_min_op()

    nc = bacc.Bacc(
        "TRN2", target_bir_lowering=False, debug=False, num_devices=N_CORES)
    f32 = mybir.dt.float32
    f16 = mybir.dt.float16
    bf16 = mybir.dt.bfloat16

    # weights: chunk c=g*4+w of cluster cl at partitions [32w, 32w+K),
    #          cols (cl*2+g)*128 ... +128
    aw_d = nc.dram_tensor("aw", [128, CPC * 2 * 128], f16, kind="ExternalInput")
    bw_d = nc.dram_tensor("bw", [128, CPC * 2 * 128], f16, kind="ExternalInput")
    # rhs replicated at the 4 partition offsets: cluster cl at cols cl*1024
    ar_d = nc.dram_tensor("ar", [128, CPC * P], f16, kind="ExternalInput")
    br_d = nc.dram_tensor("br", [128, CPC * P], f16, kind="ExternalInput")
    out_d = nc.dram_tensor("out", [128, OUT_COLS], f32, kind="ExternalOutput")

    with TileContext(nc) as tc:
        with (
            tc.tile_pool(name="io", bufs=1) as iop,
            tc.tile_pool(name="psum", bufs=2, space="PSUM") as pp,
            tc.tile_pool(name="scr", bufs=4) as sp,
        ):
            aw_t = iop.tile([128, CPC * 2 * 128], f16)
            bw_t = iop.tile([128, CPC * 2 * 128], f16)
            ar_t = iop.tile([128, CPC * P], f16)
            br_t = iop.tile([128, CPC * P], f16)
            # first slice small (1 cluster) so compute starts ASAP, then
            # the rest in growing pieces; dir-0 operands (aw/br) first
            bounds = [0, 1, 3, 6, 10, CPC]
            for q in range(len(bounds) - 1):
                ws = slice(bounds[q] * 2 * 128, bounds[q + 1] * 2 * 128)
                rs = slice(bounds[q] * P, bounds[q + 1] * P)
                nc.sync.dma_start(out=aw_t[:, ws], in_=aw_d[:, ws])
                if q == 0:
                    nc.sync.dma_start(out=br_t[:, 512:1024],
                                      in_=br_d[:, 512:1024])
                    nc.sync.dma_start(out=br_t[:, 0:512], in_=br_d[:, 0:512])
                else:
                    nc.sync.dma_start(out=br_t[:, rs], in_=br_d[:, rs])
            for q in range(len(bounds) - 1):
                ws = slice(bounds[q] * 2 * 128, bounds[q + 1] * 2 * 128)
                rs = slice(bounds[q] * P, bounds[q + 1] * P)
                nc.sync.dma_start(out=bw_t[:, ws], in_=bw_d[:, ws])
                nc.sync.dma_start(out=ar_t[:, rs], in_=ar_d[:, rs])
            mins_t = iop.tile([128, OUT_COLS], f32)
            # persistent 8-bank PSUM tile; bank h*4+w holds (chunk w, jhalf h)
            ps = pp.tile([128, 8, 512], f32, name="ps", bufs=1)

            for dirn in range(2):
                if dirn == 1:
                    nc.sync.dma_start(out=out_d[:, 0:128],
                                      in_=mins_t[:, 0:128])
                wt, rt = (aw_t, br_t) if dirn == 0 else (bw_t, ar_t)
                for cl in range(CPC):
                    for g in range(2):
                        # h1 first: its banks are freed by the (early) ACT
                        # cast of the previous wave, so PE never stalls on
                        # the custom-op drain of banks 0-3.
                        for h in (1, 0):
                            for w in range(4):
                                lhsT = wt[32 * w:32 * w + K,
                                          (cl * 2 + g) * 128:
                                          (cl * 2 + g + 1) * 128]
                                rhs = rt[32 * w:32 * w + K,
                                         cl * P + h * 512:cl * P + h * 512 + 512]
                                nc.tensor.matmul(
                                    ps[:, h * 4 + w, :],
                                    lhsT, rhs, start=True, stop=True,
                                    tile_position=(32 * w, 0))
                        e_w = sp.tile([128, 4, 512], bf16, tag="e1",
                                      bufs=3, name="e_w")
                        # two half-casts: customs c0/c1 unblock ~800ns
                        # earlier than with a single 4-bank cast
                        nc.scalar.copy(out=e_w[:, 0:2, :], in_=ps[:, 4:6, :])
                        nc.scalar.copy(out=e_w[:, 2:4, :], in_=ps[:, 6:8, :])
                        for w in range(4):
                            c = g * 4 + w
                            col = (dirn * CPC + cl) * ICH + c
                            scb = sp.tile([128, 512], bf16, tag="scb",
                                          name="scb")
                            nc.vector._custom_dve(
                                fold_min, out=scb[:],
                                in0=ps[:, w, :],
                                in1=e_w[:, w, :],
                                s0=3.0e38,
                                accum_out=mins_t[:, col:col + 1])

            nc.sync.dma_start(out=out_d[:, 128:256],
                              in_=mins_t[:, 128:256])
    nc.compile()
    return nc


def _split(x):
    hi = x.astype(np.float16)
    lo = (x - hi.astype(np.float32)).astype(np.float16)
    return hi, lo


def _prep(input_points, output_points):
    a = np.ascontiguousarray(input_points, dtype=np.float32).reshape(C, P, DIM)
    b = np.ascontiguousarray(output_points, dtype=np.float32).reshape(C, P, DIM)
    aa = np.einsum("cpd,cpd->cp", a, a).astype(np.float32)
    bb = np.einsum("cpd,cpd->cp", b, b).astype(np.float32)

    at = a.transpose(0, 2, 1)            # [C,3,P]
    bt2 = -2.0 * b.transpose(0, 2, 1)    # [C,3,P]
    ah, al = _split(at)
    bh, bl = _split(bt2)
    aah, aal = _split(aa)
    bbh, bbl = _split(bb)

    # d = sum_k A[k,i] * B[k,j]
    a_op = np.empty((C, K, P), np.float16)
    a_op[:, 0:3] = ah
    a_op[:, 3:6] = al
    a_op[:, 6:9] = ah
    a_op[:, 9:11] = 1.0
    a_op[:, 11] = aah
    a_op[:, 12] = aal

    b_op = np.empty((C, K, P), np.float16)
    b_op[:, 0:3] = bh
    b_op[:, 3:6] = bh
    b_op[:, 6:9] = bl
    b_op[:, 9] = bbh
    b_op[:, 10] = bbl
    b_op[:, 11:13] = 1.0

    def weights_layout(op_sl):
        # op_sl: [CPC, K, P] -> [128, CPC*2*128]
        # chunk c = g*4+w at partitions 32w..32w+K, cols (cl*2+g)*128
        v = op_sl.reshape(CPC, K, 2, 4, 128)     # cl, k, g, w, x
        out = np.zeros((4, 32, CPC, 2, 128), np.float16)
        out[:, :K] = v.transpose(3, 1, 0, 2, 4)  # w, k, cl, g, x
        return out.reshape(128, CPC * 2 * 128)

    def rhs_layout(op_sl):
        # op_sl: [CPC, K, P] -> [128, CPC*P] replicated at 4 offsets
        out = np.zeros((4, 32, CPC, P), np.float16)
        out[:, :K] = op_sl.transpose(1, 0, 2)[None]
        return out.reshape(128, CPC * P)

    in_maps = []
    for i in range(N_CORES):
        sl = slice(i * CPC, (i + 1) * CPC)
        in_maps.append({
            "aw": weights_layout(a_op[sl]),
            "bw": weights_layout(b_op[sl]),
            "ar": rhs_layout(a_op[sl]),
            "br": rhs_layout(b_op[sl]),
        })
    return in_maps


def run(inputs, trace=False, trace_kwargs=None):
    from concourse.bass_utils import run_bass_kernel_spmd

    if "nc" not in _cache:
        _cache["nc"] = _build()
    nc = _cache["nc"]

    in_maps = _prep(inputs["input_points"], inputs["output_points"])
    res = run_bass_kernel_spmd(
        nc, in_maps, list(range(N_CORES)),
        trace=trace, **(trace_kwargs or {}))

    per_cluster = np.concatenate([
        res.results[i]["out"].reshape(128, 2, CPC, ICH).sum(
            axis=(0, 1, 3), dtype=np.float64)
        for i in range(N_CORES)
    ])  # [C]

    nb = int(np.max(inputs["input_clusters"]))
    mask = np.arange(C) < nb
    total = np.float32(per_cluster[mask].sum())
    return np.array(total, dtype=np.float32), res


def kernel(input_points, input_clusters, output_points, output_clusters):
    loss, _ = run({
        "input_points": input_points,
        "input_clusters": input_clusters,
        "output_points": output_points,
        "output_clusters": output_clusters,
    })
    return loss



# revision 5
# speedup vs baseline: 2.1065x; 2.1065x over previous
"""Chamfer distance v3: kd-leaf candidate pruning + paged scan-min.

Host: per cluster and direction, kd-sort the query cloud into 8 spatial
leaves of 128 points; for each leaf pick T=256 candidate points of the
other cloud by leaf-box distance (+ orphan rescue into the 2 nearest
leaves).  Device: per (dir, cluster): 8 matmuls [K=14 x 256] compute the
leaf-vs-candidates distance tiles into PSUM (4 banks), with a baked-in
per-leaf bias row of -128*L so the 8 tiles are strictly decreasing by
leaf index.  ACT casts the odd column-halves to SBUF f32; one custom DVE
scan-min op (out = running min of min(in0, in1)) streams the even halves
from PSUM and the odd halves from SBUF; the value at each page end is
that leaf's per-row min (minus the known bias).  A tiny DVE op extracts
the 8 page-end columns.  Host sums, adds back the bias constant, masks
the top cluster id.
"""

import numpy as np

C = 128
P = 1024
DIM = 3
K = 14
N_CORES = 8
CPC = C // N_CORES   # 16 clusters per core
NLEAF = 8
LEAF = 128
T = 256              # candidates per leaf
BIAS = 128.0         # per-leaf page bias (> max possible distance)
OUT_COLS = 2 * CPC * NLEAF  # 256

_cache = {}


def _get_scan_min_op():
    """Register (once) a custom DVE op: out = running-min over the free
    dim of min(in0, in1), init s0."""
    from concourse.dve_spec import Spec, Src0, Src1, C0, minn, scan, AluOp
    from concourse import dve_ops as dvo
    from concourse.dve_table_gen import dve_ver_for

    name = "SCAN_MIN2_ANT"
    for op in dvo.OPS:
        if op.name == name:
            return op
    op = dvo.DveOp(
        name,
        Spec(body=scan(AluOp.MIN, minn(Src0, Src1), init=C0)),
        subdim=False,
        uops_sha={},
    )
    dvo.OPS.append(op)
    dvo.CUSTOM_DVE_SPECS[name] = op.spec
    dvo._SUB_OPCODE_FOR_NAME[name] = max(dvo._SUB_OPCODE_FOR_NAME.values()) + 1
    ver = dve_ver_for("TRN2")
    try:
        op.compile(ver)
    except ValueError as e:
        got = str(e).split(f"{ver}: ")[1].split(" ≠")[0].strip()
        op.uops_sha[ver] = got
    op.compile(ver)
    return op


def _build():
    import concourse.bacc as bacc
    import concourse.mybir as mybir
    from concourse.tile import TileContext

    scan_min = _get_scan_min_op()

    nc = bacc.Bacc(
        "TRN2", target_bir_lowering=False, debug=False, num_devices=N_CORES)
    f32 = mybir.dt.float32
    f16 = mybir.dt.float16

    # weights (queries): quadrant w rows [14] at partitions 32w,
    #   cols cl*256 + (L//4)*128 + i   for leaf L with L%4 == w
    # rhs (candidates): cols cl*512 + (L//4)*256 + slot
    wd = [nc.dram_tensor(f"w{d}", [4 * K, CPC * 2 * LEAF], f16,
                         kind="ExternalInput") for d in range(2)]
    rd = [nc.dram_tensor(f"r{d}", [4 * K, CPC * 2 * T], f16,
                         kind="ExternalInput") for d in range(2)]
    out_d = nc.dram_tensor("out", [128, OUT_COLS], f32, kind="ExternalOutput")

    with TileContext(nc) as tc:
        with (
            tc.tile_pool(name="io", bufs=2) as iop,
            tc.tile_pool(name="psum", bufs=2, space="PSUM") as pp,
            tc.tile_pool(name="ecast", bufs=3) as ep,
            tc.tile_pool(name="scr", bufs=3) as sp,
            tc.tile_pool(name="mout", bufs=1) as mp,
        ):
            mins_t = mp.tile([128, OUT_COLS], f32)
            for d in range(2):
                w_t = iop.tile([128, CPC * 2 * LEAF], f16, tag="w")
                r_t = iop.tile([128, CPC * 2 * T], f16, tag="r")
                # first two clusters' slices first so compute starts ASAP
                wb = [0, 2 * 2 * LEAF, CPC * 2 * LEAF]
                rb = [0, 2 * 2 * T, CPC * 2 * T]
                for q in range(len(wb) - 1):
                    for w in range(4):
                        nc.sync.dma_start(
                            out=w_t[32 * w:32 * w + K, wb[q]:wb[q + 1]],
                            in_=wd[d][K * w:K * w + K, wb[q]:wb[q + 1]])
                        nc.sync.dma_start(
                            out=r_t[32 * w:32 * w + K, rb[q]:rb[q + 1]],
                            in_=rd[d][K * w:K * w + K, rb[q]:rb[q + 1]])
                for cl in range(CPC):
                    scr = sp.tile([128, NLEAF, T // 2], f32, name="s")
                    for h in range(2):
                        ps = pp.tile([128, 4, 512], f32, name="ps")
                        for w in range(4):
                            lhsT = w_t[32 * w:32 * w + K,
                                       cl * 2 * LEAF + h * LEAF:
                                       cl * 2 * LEAF + (h + 1) * LEAF]
                            rhs = r_t[32 * w:32 * w + K,
                                      cl * 2 * T + h * T:
                                      cl * 2 * T + (h + 1) * T]
                            nc.tensor.matmul(ps[:, w, 0:T], lhsT, rhs,
                                             start=True, stop=True,
                                             tile_position=(32 * w, 0))
                        e_t = ep.tile([128, 4, T // 2], f32, name="e")
                        nc.scalar.copy(out=e_t[:], in_=ps[:, :, T // 2:T])
                        nc.vector._custom_dve(
                            scan_min, out=scr[:, 4 * h:4 * h + 4, :],
                            in0=ps[:, :, 0:T // 2], in1=e_t[:], s0=3.0e38)
                    col = (d * CPC + cl) * NLEAF
                    nc.vector.tensor_scalar_add(
                        out=mins_t[:, col:col + NLEAF],
                        in0=scr[:, :, T // 2 - 1:T // 2], scalar1=0.0)
                nc.sync.dma_start(
                    out=out_d[:, d * 128:(d + 1) * 128],
                    in_=mins_t[:, d * 128:(d + 1) * 128])
    nc.compile()
    return nc


def _split(x):
    hi = x.astype(np.float16)
    lo = (x - hi.astype(np.float32)).astype(np.float16)
    return hi, lo


def _kd_leaves(pts):
    """pts [P,3] f32 -> permutation so each consecutive LEAF block is a
    kd leaf (median split along longest extent)."""
    out = []

    def rec(ids):
        if len(ids) <= LEAF:
            out.append(ids)
            return
        sub = pts[ids]
        ext = sub.max(0) - sub.min(0)
        dim = int(np.argmax(ext))
        k = len(ids) // 2
        part = np.argpartition(sub[:, dim], k)
        rec(ids[part[:k]])
        rec(ids[part[k:]])

    rec(np.arange(len(pts)))
    return np.concatenate(out)


def _cand_lists(xs, y):
    """xs [NLEAF, LEAF, 3] sorted queries; y [P,3] candidates.
    Returns [NLEAF, T] candidate indices (box-distance top-T, orphans
    forced into their 2 nearest leaves)."""
    lo = xs.min(1)[:, None, :]
    hi = xs.max(1)[:, None, :]
    dd = np.maximum(lo - y[None], 0.0) + np.maximum(y[None] - hi, 0.0)
    boxd = (dd * dd).sum(-1)                      # [NLEAF, P]
    part = np.argpartition(boxd, T - 1, axis=1)[:, :T]
    # order each list by box distance so rescue replaces the worst slots
    rows = np.arange(NLEAF)[:, None]
    order = np.argsort(boxd[rows, part], axis=1)
    lists = part[rows, order]
    present = np.zeros(P, bool)
    present[lists.ravel()] = True
    orphans = np.where(~present)[0]
    if len(orphans):
        nearest = np.argsort(boxd[:, orphans], axis=0)[:2]  # [2, n]
        back = [T - 1] * NLEAF
        for r in range(2):
            for j, L in zip(orphans, nearest[r]):
                lists[L, back[L]] = j
                back[L] -= 1
    return lists


def _prep(input_points, output_points):
    a = np.ascontiguousarray(input_points, dtype=np.float32).reshape(C, P, DIM)
    b = np.ascontiguousarray(output_points, dtype=np.float32).reshape(C, P, DIM)

    # layouts per direction: w_flat [C, 2, 4, K, LEAF], r_flat [C, 2, 4, K, T]
    w_flat = np.zeros((2, C, 2, 4, K, LEAF), np.float16)
    r_flat = np.zeros((2, C, 2, 4, K, T), np.float16)
    # bias descends within each 4-leaf half-cluster unit (page index w)
    bias_row = np.empty((2, 4, T), np.float16)
    for h in range(2):
        for w in range(4):
            bias_row[h, w, :] = -BIAS * w

    for c in range(C):
        for d, (q, y) in enumerate(((a[c], b[c]), (b[c], a[c]))):
            perm = _kd_leaves(q)
            xs = q[perm].reshape(NLEAF, LEAF, DIM)
            lists = _cand_lists(xs, y)
            cands = y[lists]                       # [NLEAF, T, 3]

            qt = xs.transpose(0, 2, 1)             # [NLEAF, 3, LEAF]
            qh, ql = _split(qt)
            qq = (xs * xs).sum(-1)                 # [NLEAF, LEAF]
            qqh, qql = _split(qq)

            ct = -2.0 * cands.transpose(0, 2, 1)   # [NLEAF, 3, T]
            ch, cl_ = _split(ct)
            cc = (cands * cands).sum(-1)           # [NLEAF, T]
            cch, ccl = _split(cc)

            wv = np.empty((NLEAF, K, LEAF), np.float16)
            wv[:, 0:3] = qh
            wv[:, 3:6] = ql
            wv[:, 6:9] = qh
            wv[:, 9:11] = 1.0
            wv[:, 11] = qqh
            wv[:, 12] = qql
            wv[:, 13] = 1.0

            rv = np.empty((NLEAF, K, T), np.float16)
            rv[:, 0:3] = ch
            rv[:, 3:6] = ch
            rv[:, 6:9] = cl_
            rv[:, 9] = cch
            rv[:, 10] = ccl
            rv[:, 11:13] = 1.0
            rv[:, 13] = bias_row.reshape(NLEAF, T)

            w_flat[d, c] = wv.reshape(2, 4, K, LEAF)
            r_flat[d, c] = rv.reshape(2, 4, K, T)

    in_maps = []
    for i in range(N_CORES):
        sl = slice(i * CPC, (i + 1) * CPC)
        m = {}
        for d in range(2):
            # [cl, h, w, k, x] -> [w, k, cl, h, x]
            m[f"w{d}"] = np.ascontiguousarray(
                w_flat[d, sl].transpose(2, 3, 0, 1, 4)).reshape(
                    4 * K, CPC * 2 * LEAF)
            m[f"r{d}"] = np.ascontiguousarray(
                r_flat[d, sl].transpose(2, 3, 0, 1, 4)).reshape(
                    4 * K, CPC * 2 * T)
        in_maps.append(m)
    return in_maps


def run(inputs, trace=False, trace_kwargs=None):
    from concourse.bass_utils import run_bass_kernel_spmd

    if "nc" not in _cache:
        _cache["nc"] = _build()
    nc = _cache["nc"]

    in_maps = _prep(inputs["input_points"], inputs["output_points"])
    res = run_bass_kernel_spmd(
        nc, in_maps, list(range(N_CORES)),
        trace=trace, **(trace_kwargs or {}))

    # out[:, (d*CPC+cl)*NLEAF + L] = leaf min - BIAS*(L%4) per partition
    bias_const = 128.0 * BIAS * 2 * (0 + 1 + 2 + 3)  # per (d, cl)
    per_cluster = np.concatenate([
        res.results[i]["out"].reshape(128, 2, CPC, NLEAF).sum(
            axis=(0, 1, 3), dtype=np.float64)
        for i in range(N_CORES)
    ]) + 2 * bias_const  # [C]

    nb = int(np.max(inputs["input_clusters"]))
    mask = np.arange(C) < nb
    total = np.float32(per_cluster[mask].sum())
    return np.array(total, dtype=np.float32), res


def kernel(input_points, input_clusters, output_points, output_clusters):
    loss, _ = run({
        "input_points": input_points,
        "input_clusters": input_clusters,
        "output_points": output_points,
        "output_clusters": output_clusters,
    })
    return loss


# revision 7
# speedup vs baseline: 2.4650x; 1.1702x over previous
"""Chamfer distance v3: kd-leaf candidate pruning + paged scan-min.

Host: per cluster and direction, kd-sort the query cloud into 8 spatial
leaves of 128 points; for each leaf pick T=256 candidate points of the
other cloud by leaf-box distance (+ orphan rescue into the 2 nearest
leaves).  Device: per (dir, cluster): 8 matmuls [K=14 x 256] compute the
leaf-vs-candidates distance tiles into PSUM (4 banks), with a baked-in
per-leaf bias row of -128*L so the 8 tiles are strictly decreasing by
leaf index.  ACT casts the odd column-halves to SBUF f32; one custom DVE
scan-min op (out = running min of min(in0, in1)) streams the even halves
from PSUM and the odd halves from SBUF; the value at each page end is
that leaf's per-row min (minus the known bias).  A tiny DVE op extracts
the 8 page-end columns.  Host sums, adds back the bias constant, masks
the top cluster id.
"""

import numpy as np

C = 128
P = 1024
DIM = 3
K = 14
N_CORES = 8
CPC = C // N_CORES   # 16 clusters per core
NLEAF = 8
LEAF = 128
T = 256              # candidates per leaf
BIAS = 128.0         # per-leaf page bias (> max possible distance)
OUT_COLS = 2 * CPC * NLEAF  # 256

_cache = {}


def _get_scan_min_op():
    """Register (once) a custom DVE op: out = running-min over the free
    dim of min(in0, in1), init s0."""
    from concourse.dve_spec import Spec, Src0, Src1, C0, minn, scan, AluOp
    from concourse import dve_ops as dvo
    from concourse.dve_table_gen import dve_ver_for

    name = "SCAN_MIN2_ANT"
    for op in dvo.OPS:
        if op.name == name:
            return op
    op = dvo.DveOp(
        name,
        Spec(body=scan(AluOp.MIN, minn(Src0, Src1), init=C0)),
        subdim=False,
        uops_sha={},
    )
    dvo.OPS.append(op)
    dvo.CUSTOM_DVE_SPECS[name] = op.spec
    dvo._SUB_OPCODE_FOR_NAME[name] = max(dvo._SUB_OPCODE_FOR_NAME.values()) + 1
    ver = dve_ver_for("TRN2")
    try:
        op.compile(ver)
    except ValueError as e:
        got = str(e).split(f"{ver}: ")[1].split(" ≠")[0].strip()
        op.uops_sha[ver] = got
    op.compile(ver)
    return op


def _build():
    import concourse.bacc as bacc
    import concourse.mybir as mybir
    from concourse.tile import TileContext

    scan_min = _get_scan_min_op()

    nc = bacc.Bacc(
        "TRN2", target_bir_lowering=False, debug=False, num_devices=N_CORES)
    f32 = mybir.dt.float32
    f16 = mybir.dt.float16

    # weights (queries): quadrant w rows [14] at partitions 32w,
    #   cols cl*256 + (L//4)*128 + i   for leaf L with L%4 == w
    # rhs (candidates): cols cl*512 + (L//4)*256 + slot
    wd = [nc.dram_tensor(f"w{d}", [4 * K, CPC * 2 * LEAF], f16,
                         kind="ExternalInput") for d in range(2)]
    rd = [nc.dram_tensor(f"r{d}", [4 * K, CPC * 2 * T], f16,
                         kind="ExternalInput") for d in range(2)]
    out_d = nc.dram_tensor("out", [128, OUT_COLS], f32, kind="ExternalOutput")

    with TileContext(nc) as tc:
        with (
            tc.tile_pool(name="io", bufs=2) as iop,
            tc.tile_pool(name="psum", bufs=4, space="PSUM") as pp,
            tc.tile_pool(name="ecast", bufs=5) as ep,
            tc.tile_pool(name="scr", bufs=3) as sp,
            tc.tile_pool(name="mout", bufs=1) as mp,
        ):
            mins_t = mp.tile([128, OUT_COLS], f32)
            for d in range(2):
                w_t = iop.tile([128, CPC * 2 * LEAF], f16, tag="w")
                r_t = iop.tile([128, CPC * 2 * T], f16, tag="r")
                # first two clusters' slices first so compute starts ASAP
                wb = [0, 2 * 2 * LEAF, CPC * 2 * LEAF]
                rb = [0, 2 * 2 * T, CPC * 2 * T]
                for q in range(len(wb) - 1):
                    for w in range(4):
                        nc.sync.dma_start(
                            out=w_t[32 * w:32 * w + K, wb[q]:wb[q + 1]],
                            in_=wd[d][K * w:K * w + K, wb[q]:wb[q + 1]])
                        nc.sync.dma_start(
                            out=r_t[32 * w:32 * w + K, rb[q]:rb[q + 1]],
                            in_=rd[d][K * w:K * w + K, rb[q]:rb[q + 1]])
                for cl in range(CPC):
                    scr = sp.tile([128, NLEAF, T // 2], f32, name="s")
                    for u in range(4):
                        # unit u = leaves 2u, 2u+1 (quadrants (2u)%4, (2u+1)%4)
                        ps = pp.tile([128, 2, 512], f32, name="ps")
                        for j in range(2):
                            L = 2 * u + j
                            w, h = L % 4, L // 4
                            lhsT = w_t[32 * w:32 * w + K,
                                       cl * 2 * LEAF + h * LEAF:
                                       cl * 2 * LEAF + (h + 1) * LEAF]
                            rhs = r_t[32 * w:32 * w + K,
                                      cl * 2 * T + h * T:
                                      cl * 2 * T + (h + 1) * T]
                            nc.tensor.matmul(ps[:, j, 0:T], lhsT, rhs,
                                             start=True, stop=True,
                                             tile_position=(32 * w, 0))
                        e_t = ep.tile([128, 2, T // 2], f32, name="e")
                        nc.scalar.copy(out=e_t[:], in_=ps[:, :, T // 2:T])
                        nc.vector._custom_dve(
                            scan_min, out=scr[:, 2 * u:2 * u + 2, :],
                            in0=ps[:, :, 0:T // 2], in1=e_t[:], s0=3.0e38)
                    col = (d * CPC + cl) * NLEAF
                    nc.vector.tensor_scalar_add(
                        out=mins_t[:, col:col + NLEAF],
                        in0=scr[:, :, T // 2 - 1:T // 2], scalar1=0.0)
                nc.sync.dma_start(
                    out=out_d[:, d * 128:(d + 1) * 128],
                    in_=mins_t[:, d * 128:(d + 1) * 128])
    nc.compile()
    return nc


def _split(x):
    hi = x.astype(np.float16)
    lo = (x - hi.astype(np.float32)).astype(np.float16)
    return hi, lo


def _kd_leaves(pts):
    """pts [P,3] f32 -> permutation so each consecutive LEAF block is a
    kd leaf (median split along longest extent)."""
    out = []

    def rec(ids):
        if len(ids) <= LEAF:
            out.append(ids)
            return
        sub = pts[ids]
        ext = sub.max(0) - sub.min(0)
        dim = int(np.argmax(ext))
        k = len(ids) // 2
        part = np.argpartition(sub[:, dim], k)
        rec(ids[part[:k]])
        rec(ids[part[k:]])

    rec(np.arange(len(pts)))
    return np.concatenate(out)


def _cand_lists(xs, y):
    """xs [NLEAF, LEAF, 3] sorted queries; y [P,3] candidates.
    Returns [NLEAF, T] candidate indices (box-distance top-T, orphans
    forced into their 2 nearest leaves)."""
    lo = xs.min(1)[:, None, :]
    hi = xs.max(1)[:, None, :]
    dd = np.maximum(lo - y[None], 0.0) + np.maximum(y[None] - hi, 0.0)
    boxd = (dd * dd).sum(-1)                      # [NLEAF, P]
    part = np.argpartition(boxd, T - 1, axis=1)[:, :T]
    # order each list by box distance so rescue replaces the worst slots
    rows = np.arange(NLEAF)[:, None]
    order = np.argsort(boxd[rows, part], axis=1)
    lists = part[rows, order]
    present = np.zeros(P, bool)
    present[lists.ravel()] = True
    orphans = np.where(~present)[0]
    if len(orphans):
        nearest = np.argsort(boxd[:, orphans], axis=0)[:2]  # [2, n]
        back = [T - 1] * NLEAF
        for r in range(2):
            for j, L in zip(orphans, nearest[r]):
                lists[L, back[L]] = j
                back[L] -= 1
    return lists


def _prep(input_points, output_points):
    a = np.ascontiguousarray(input_points, dtype=np.float32).reshape(C, P, DIM)
    b = np.ascontiguousarray(output_points, dtype=np.float32).reshape(C, P, DIM)

    # layouts per direction: w_flat [C, 2, 4, K, LEAF], r_flat [C, 2, 4, K, T]
    w_flat = np.zeros((2, C, 2, 4, K, LEAF), np.float16)
    r_flat = np.zeros((2, C, 2, 4, K, T), np.float16)
    # bias descends within each 2-leaf unit (page index L%2 = w%2)
    bias_row = np.empty((2, 4, T), np.float16)
    for h in range(2):
        for w in range(4):
            bias_row[h, w, :] = -BIAS * (w % 2)

    for c in range(C):
        for d, (q, y) in enumerate(((a[c], b[c]), (b[c], a[c]))):
            perm = _kd_leaves(q)
            xs = q[perm].reshape(NLEAF, LEAF, DIM)
            lists = _cand_lists(xs, y)
            cands = y[lists]                       # [NLEAF, T, 3]

            qt = xs.transpose(0, 2, 1)             # [NLEAF, 3, LEAF]
            qh, ql = _split(qt)
            qq = (xs * xs).sum(-1)                 # [NLEAF, LEAF]
            qqh, qql = _split(qq)

            ct = -2.0 * cands.transpose(0, 2, 1)   # [NLEAF, 3, T]
            ch, cl_ = _split(ct)
            cc = (cands * cands).sum(-1)           # [NLEAF, T]
            cch, ccl = _split(cc)

            wv = np.empty((NLEAF, K, LEAF), np.float16)
            wv[:, 0:3] = qh
            wv[:, 3:6] = ql
            wv[:, 6:9] = qh
            wv[:, 9:11] = 1.0
            wv[:, 11] = qqh
            wv[:, 12] = qql
            wv[:, 13] = 1.0

            rv = np.empty((NLEAF, K, T), np.float16)
            rv[:, 0:3] = ch
            rv[:, 3:6] = ch
            rv[:, 6:9] = cl_
            rv[:, 9] = cch
            rv[:, 10] = ccl
            rv[:, 11:13] = 1.0
            rv[:, 13] = bias_row.reshape(NLEAF, T)

            w_flat[d, c] = wv.reshape(2, 4, K, LEAF)
            r_flat[d, c] = rv.reshape(2, 4, K, T)

    in_maps = []
    for i in range(N_CORES):
        sl = slice(i * CPC, (i + 1) * CPC)
        m = {}
        for d in range(2):
            # [cl, h, w, k, x] -> [w, k, cl, h, x]
            m[f"w{d}"] = np.ascontiguousarray(
                w_flat[d, sl].transpose(2, 3, 0, 1, 4)).reshape(
                    4 * K, CPC * 2 * LEAF)
            m[f"r{d}"] = np.ascontiguousarray(
                r_flat[d, sl].transpose(2, 3, 0, 1, 4)).reshape(
                    4 * K, CPC * 2 * T)
        in_maps.append(m)
    return in_maps


def run(inputs, trace=False, trace_kwargs=None):
    from concourse.bass_utils import run_bass_kernel_spmd

    if "nc" not in _cache:
        _cache["nc"] = _build()
    nc = _cache["nc"]

    in_maps = _prep(inputs["input_points"], inputs["output_points"])
    res = run_bass_kernel_spmd(
        nc, in_maps, list(range(N_CORES)),
        trace=trace, **(trace_kwargs or {}))

    # out[:, (d*CPC+cl)*NLEAF + L] = leaf min - BIAS*(L%2) per partition
    bias_const = 128.0 * BIAS * 4  # per (d, cl)
    per_cluster = np.concatenate([
        res.results[i]["out"].reshape(128, 2, CPC, NLEAF).sum(
            axis=(0, 1, 3), dtype=np.float64)
        for i in range(N_CORES)
    ]) + 2 * bias_const  # [C]

    nb = int(np.max(inputs["input_clusters"]))
    mask = np.arange(C) < nb
    total = np.float32(per_cluster[mask].sum())
    return np.array(total, dtype=np.float32), res


def kernel(input_points, input_clusters, output_points, output_clusters):
    loss, _ = run({
        "input_points": input_points,
        "input_clusters": input_clusters,
        "output_points": output_points,
        "output_clusters": output_clusters,
    })
    return loss


# revision 8
# speedup vs baseline: 2.6751x; 1.0852x over previous
"""Chamfer distance v3: kd-leaf candidate pruning + paged scan-min.

Host: per cluster and direction, kd-sort the query cloud into 8 spatial
leaves of 128 points; for each leaf pick T=256 candidate points of the
other cloud by leaf-box distance (+ orphan rescue into the 2 nearest
leaves).  Device: per (dir, cluster): 8 matmuls [K=14 x 256] compute the
leaf-vs-candidates distance tiles into PSUM (4 banks), with a baked-in
per-leaf bias row of -128*L so the 8 tiles are strictly decreasing by
leaf index.  ACT casts the odd column-halves to SBUF f32; one custom DVE
scan-min op (out = running min of min(in0, in1)) streams the even halves
from PSUM and the odd halves from SBUF; the value at each page end is
that leaf's per-row min (minus the known bias).  A tiny DVE op extracts
the 8 page-end columns.  Host sums, adds back the bias constant, masks
the top cluster id.
"""

import numpy as np

C = 128
P = 1024
DIM = 3
K = 14
N_CORES = 8
CPC = C // N_CORES   # 16 clusters per core
NLEAF = 8
LEAF = 128
T = 256              # candidates per leaf
BIAS = 128.0         # per-leaf page bias (> max possible distance)
OUT_COLS = 2 * CPC * NLEAF  # 256

_cache = {}


def _get_scan_min_op():
    """Register (once) a custom DVE op: out = running-min over the free
    dim of min(in0, in1), init s0."""
    from concourse.dve_spec import Spec, Src0, Src1, C0, minn, scan, AluOp
    from concourse import dve_ops as dvo
    from concourse.dve_table_gen import dve_ver_for

    name = "SCAN_MIN2_ANT"
    for op in dvo.OPS:
        if op.name == name:
            return op
    op = dvo.DveOp(
        name,
        Spec(body=scan(AluOp.MIN, minn(Src0, Src1), init=C0)),
        subdim=False,
        uops_sha={},
    )
    dvo.OPS.append(op)
    dvo.CUSTOM_DVE_SPECS[name] = op.spec
    dvo._SUB_OPCODE_FOR_NAME[name] = max(dvo._SUB_OPCODE_FOR_NAME.values()) + 1
    ver = dve_ver_for("TRN2")
    try:
        op.compile(ver)
    except ValueError as e:
        got = str(e).split(f"{ver}: ")[1].split(" ≠")[0].strip()
        op.uops_sha[ver] = got
    op.compile(ver)
    return op


def _build():
    import concourse.bacc as bacc
    import concourse.mybir as mybir
    from concourse.tile import TileContext

    scan_min = _get_scan_min_op()

    nc = bacc.Bacc(
        "TRN2", target_bir_lowering=False, debug=False, num_devices=N_CORES)
    f32 = mybir.dt.float32
    f16 = mybir.dt.float16

    # weights (queries): quadrant w rows [14] at partitions 32w,
    #   cols cl*256 + (L//4)*128 + i   for leaf L with L%4 == w
    # rhs (candidates): cols cl*512 + (L//4)*256 + slot
    wd = [nc.dram_tensor(f"w{d}", [4 * K, CPC * 2 * LEAF], f16,
                         kind="ExternalInput") for d in range(2)]
    rd = [nc.dram_tensor(f"r{d}", [4 * K, CPC * 2 * T], f16,
                         kind="ExternalInput") for d in range(2)]
    out_d = nc.dram_tensor("out", [128, OUT_COLS], f32, kind="ExternalOutput")

    with TileContext(nc) as tc:
        with (
            tc.tile_pool(name="io", bufs=2) as iop,
            tc.tile_pool(name="psum", bufs=4, space="PSUM") as pp,
            tc.tile_pool(name="ecast", bufs=5) as ep,
            tc.tile_pool(name="scr", bufs=3) as sp,
            tc.tile_pool(name="mout", bufs=1) as mp,
        ):
            mins_t = mp.tile([128, OUT_COLS], f32)
            for d in range(2):
                w_t = iop.tile([128, CPC * 2 * LEAF], f16, tag="w")
                r_t = iop.tile([128, CPC * 2 * T], f16, tag="r")
                # growing slices so compute starts ASAP; weights on the SP
                # DMA queue, rhs on the GpSimd queue (parallel transfer)
                cb = [0, 1, 2, 4, 8, CPC]
                for q in range(len(cb) - 1):
                    ws = slice(cb[q] * 2 * LEAF, cb[q + 1] * 2 * LEAF)
                    rs = slice(cb[q] * 2 * T, cb[q + 1] * 2 * T)
                    for w in range(4):
                        nc.sync.dma_start(
                            out=w_t[32 * w:32 * w + K, ws],
                            in_=wd[d][K * w:K * w + K, ws])
                        nc.gpsimd.dma_start(
                            out=r_t[32 * w:32 * w + K, rs],
                            in_=rd[d][K * w:K * w + K, rs])
                for cl in range(CPC):
                    scr = sp.tile([128, NLEAF, T // 2], f32, name="s")
                    for u in range(4):
                        # unit u = leaves 2u, 2u+1 (quadrants (2u)%4, (2u+1)%4)
                        ps = pp.tile([128, 2, 512], f32, name="ps")
                        for j in range(2):
                            L = 2 * u + j
                            w, h = L % 4, L // 4
                            lhsT = w_t[32 * w:32 * w + K,
                                       cl * 2 * LEAF + h * LEAF:
                                       cl * 2 * LEAF + (h + 1) * LEAF]
                            rhs = r_t[32 * w:32 * w + K,
                                      cl * 2 * T + h * T:
                                      cl * 2 * T + (h + 1) * T]
                            nc.tensor.matmul(ps[:, j, 0:T], lhsT, rhs,
                                             start=True, stop=True,
                                             tile_position=(32 * w, 0))
                        e_t = ep.tile([128, 2, T // 2], f32, name="e")
                        nc.scalar.copy(out=e_t[:], in_=ps[:, :, T // 2:T])
                        nc.vector._custom_dve(
                            scan_min, out=scr[:, 2 * u:2 * u + 2, :],
                            in0=ps[:, :, 0:T // 2], in1=e_t[:], s0=3.0e38)
                    col = (d * CPC + cl) * NLEAF
                    nc.vector.tensor_scalar_add(
                        out=mins_t[:, col:col + NLEAF],
                        in0=scr[:, :, T // 2 - 1:T // 2], scalar1=0.0)
                nc.sync.dma_start(
                    out=out_d[:, d * 128:(d + 1) * 128],
                    in_=mins_t[:, d * 128:(d + 1) * 128])
    nc.compile()
    return nc


def _split(x):
    hi = x.astype(np.float16)
    lo = (x - hi.astype(np.float32)).astype(np.float16)
    return hi, lo


def _kd_leaves(pts):
    """pts [P,3] f32 -> permutation so each consecutive LEAF block is a
    kd leaf (median split along longest extent)."""
    out = []

    def rec(ids):
        if len(ids) <= LEAF:
            out.append(ids)
            return
        sub = pts[ids]
        ext = sub.max(0) - sub.min(0)
        dim = int(np.argmax(ext))
        k = len(ids) // 2
        part = np.argpartition(sub[:, dim], k)
        rec(ids[part[:k]])
        rec(ids[part[k:]])

    rec(np.arange(len(pts)))
    return np.concatenate(out)


def _cand_lists(xs, y):
    """xs [NLEAF, LEAF, 3] sorted queries; y [P,3] candidates.
    Returns [NLEAF, T] candidate indices (box-distance top-T, orphans
    forced into their 2 nearest leaves)."""
    lo = xs.min(1)[:, None, :]
    hi = xs.max(1)[:, None, :]
    dd = np.maximum(lo - y[None], 0.0) + np.maximum(y[None] - hi, 0.0)
    boxd = (dd * dd).sum(-1)                      # [NLEAF, P]
    part = np.argpartition(boxd, T - 1, axis=1)[:, :T]
    # order each list by box distance so rescue replaces the worst slots
    rows = np.arange(NLEAF)[:, None]
    order = np.argsort(boxd[rows, part], axis=1)
    lists = part[rows, order]
    present = np.zeros(P, bool)
    present[lists.ravel()] = True
    orphans = np.where(~present)[0]
    if len(orphans):
        nearest = np.argsort(boxd[:, orphans], axis=0)[:2]  # [2, n]
        back = [T - 1] * NLEAF
        for r in range(2):
            for j, L in zip(orphans, nearest[r]):
                lists[L, back[L]] = j
                back[L] -= 1
    return lists


def _prep(input_points, output_points):
    a = np.ascontiguousarray(input_points, dtype=np.float32).reshape(C, P, DIM)
    b = np.ascontiguousarray(output_points, dtype=np.float32).reshape(C, P, DIM)

    # layouts per direction: w_flat [C, 2, 4, K, LEAF], r_flat [C, 2, 4, K, T]
    w_flat = np.zeros((2, C, 2, 4, K, LEAF), np.float16)
    r_flat = np.zeros((2, C, 2, 4, K, T), np.float16)
    # bias descends within each 2-leaf unit (page index L%2 = w%2)
    bias_row = np.empty((2, 4, T), np.float16)
    for h in range(2):
        for w in range(4):
            bias_row[h, w, :] = -BIAS * (w % 2)

    for c in range(C):
        for d, (q, y) in enumerate(((a[c], b[c]), (b[c], a[c]))):
            perm = _kd_leaves(q)
            xs = q[perm].reshape(NLEAF, LEAF, DIM)
            lists = _cand_lists(xs, y)
            cands = y[lists]                       # [NLEAF, T, 3]

            qt = xs.transpose(0, 2, 1)             # [NLEAF, 3, LEAF]
            qh, ql = _split(qt)
            qq = (xs * xs).sum(-1)                 # [NLEAF, LEAF]
            qqh, qql = _split(qq)

            ct = -2.0 * cands.transpose(0, 2, 1)   # [NLEAF, 3, T]
            ch, cl_ = _split(ct)
            cc = (cands * cands).sum(-1)           # [NLEAF, T]
            cch, ccl = _split(cc)

            wv = np.empty((NLEAF, K, LEAF), np.float16)
            wv[:, 0:3] = qh
            wv[:, 3:6] = ql
            wv[:, 6:9] = qh
            wv[:, 9:11] = 1.0
            wv[:, 11] = qqh
            wv[:, 12] = qql
            wv[:, 13] = 1.0

            rv = np.empty((NLEAF, K, T), np.float16)
            rv[:, 0:3] = ch
            rv[:, 3:6] = ch
            rv[:, 6:9] = cl_
            rv[:, 9] = cch
            rv[:, 10] = ccl
            rv[:, 11:13] = 1.0
            rv[:, 13] = bias_row.reshape(NLEAF, T)

            w_flat[d, c] = wv.reshape(2, 4, K, LEAF)
            r_flat[d, c] = rv.reshape(2, 4, K, T)

    in_maps = []
    for i in range(N_CORES):
        sl = slice(i * CPC, (i + 1) * CPC)
        m = {}
        for d in range(2):
            # [cl, h, w, k, x] -> [w, k, cl, h, x]
            m[f"w{d}"] = np.ascontiguousarray(
                w_flat[d, sl].transpose(2, 3, 0, 1, 4)).reshape(
                    4 * K, CPC * 2 * LEAF)
            m[f"r{d}"] = np.ascontiguousarray(
                r_flat[d, sl].transpose(2, 3, 0, 1, 4)).reshape(
                    4 * K, CPC * 2 * T)
        in_maps.append(m)
    return in_maps


def run(inputs, trace=False, trace_kwargs=None):
    from concourse.bass_utils import run_bass_kernel_spmd

    if "nc" not in _cache:
        _cache["nc"] = _build()
    nc = _cache["nc"]

    in_maps = _prep(inputs["input_points"], inputs["output_points"])
    res = run_bass_kernel_spmd(
        nc, in_maps, list(range(N_CORES)),
        trace=trace, **(trace_kwargs or {}))

    # out[:, (d*CPC+cl)*NLEAF + L] = leaf min - BIAS*(L%2) per partition
    bias_const = 128.0 * BIAS * 4  # per (d, cl)
    per_cluster = np.concatenate([
        res.results[i]["out"].reshape(128, 2, CPC, NLEAF).sum(
            axis=(0, 1, 3), dtype=np.float64)
        for i in range(N_CORES)
    ]) + 2 * bias_const  # [C]

    nb = int(np.max(inputs["input_clusters"]))
    mask = np.arange(C) < nb
    total = np.float32(per_cluster[mask].sum())
    return np.array(total, dtype=np.float32), res


def kernel(input_points, input_clusters, output_points, output_clusters):
    loss, _ = run({
        "input_points": input_points,
        "input_clusters": input_clusters,
        "output_points": output_points,
        "output_clusters": output_clusters,
    })
    return loss
